# revision 44
# baseline (speedup 1.0000x reference)
"""AdaptiveSparsityAttention TRN2 kernel (8 NeuronCores, SPMD data-parallel).

Problem (B=2, S=1024, D=512, H=2 heads, dh=256, hidden=128):
  q,k,v = x@Wq, x@Wk, x@Wv (split 2 heads); scores = q@k^T/16
  a_i = q_mean@W1[:dh]+b1, c_j = k_mean@W1[dh:]
  z[i,j] = W2 . relu(a_i + c_j)          (sigmoid(z+b2)>0.5  <=>  z > -b2)
  attn = softmax(mask(scores));  out = (attn@v)@Wo + bo

Sharding: 8 cores = 2 batches x 4 query-chunks of 256 rows. Each core
computes its output chunk fully locally, no collectives.

Precision structure (measured, numpy sims):
  - z MUST be fp32-exact: z std 0.022, ~46% fill, threshold 0. bf16-level
    z error flips ~0.1% of mask bits -> 5.8e-2 L2 (FAILS 2e-2 gate);
    fp16 -> 1.5e-2 (too marginal). fp32 mask matches jax ref at 1.4e-3.
  - Everything downstream of the mask is bf16-safe: bf16 q/k/v/scores/
    attn-weights/out-proj measured 4.1e-3 total vs jax ref (5x margin).
    bf16 (vs baseline's float32r) buys FWL fast weight loads on PE,
    2-4x DVE modes, and half the DMA bytes.

Performance structure (per core, from perfetto traces; 194-201us HW
at full clock (best 194823), vs 217us for the float32r baseline; the chip sometimes
drops to 2.0GHz P0 throttle under sustained benching -- all engine
durations then scale ~1.2x, check zMM med dur 592 vs 710):
  - z stream: 2 blocks x 128 steps (1 query row each) of [T-tile produce
    on DVE (~805ns) or ACT (~1136ns)] + 2 fp32 matmul calls (wsel delta
    trick, 4-way col-tiled, 4 LDW + 4 MM instrs/step, LDW ~110ns each).
    Pace ~470ns/step in blk0, T-production + LDW co-bound. Near the
    structural floor: fp32 MMs cannot share LDWs in bass (explicit
    ldweights is broken for fp32/f32r), and exact T production is
    elementwise fp32 on DVE/ACT (tensor_scalar = 2 elem/cyc/lane).
  - The z streams are kept PURE: interleaving full-array matmuls into
    the col-tiled fp32 stream drains the quadrant pipeline (~1us each;
    measured +23us/block). QKV/scores/exp/blk0-softmax run in the mid
    gap; only zp1-dependent softmax + AV + out-proj are in the tail.
  - x is column-rolled per core (host side) so the query chunk is
    j=0..255: the fp32 a-matmuls start from the first x DMA quarter and
    no separate xq transfer is needed. Softmax is invariant to the
    consistent j-permutation of keys/values/mask.
  - One DMA instruction per tensor (the DGE costs ~600ns/descriptor;
    actual transfer bandwidth ~150GB/s/queue, shared DRAM channel), on
    the 2 HW queues (sync/scalar) + gpsimd SWDGE for late-needed data.
  - The V projection + its copies and head1's ti=0 transposes are
    deferred past blk1 (first needed by AV in the tail, where PE idles
    during zp1's softmax): -9us off blk1 by shrinking the mid-gap
    DVE/ACT spill that T-starves it. Deferring the WHOLE ti=0 attention
    chain went too far: blk1 hit 69us but the tail blew up +11.5
    (the chain serializes; blk1+tail work is ~conserved, ~108us).
    12 of blk1's T-producers are pre-emitted among the mid qk units
    (body pace is LDW-bound at 429 vs 456ns/step production, so the
    pool drains only ~27ns/step; thinner interleave unblocks the mid's
    exp/copy deps sooner). The mid is ordered per-head (head h's scores
    need only qt/kt dout 2h..2h+1, emitted right after those 6 units):
    halving the sc matmuls' dependency depth cut blk1's ramp-in stall,
    74.9->69.3us. Moving head1's ti=0 softmax to the tail instead was
    a pure regression (+2.7 tail, ramp unchanged -- the ramp binder is
    ACT-queue exp/relu ordering, not the DVE softmax). The out-proj is
    split around av(1): bias+head0 accumulation fills the PE stall
    while tp(1,1) copies drain, reusing the dead zp PSUM slots; both
    o_sb copies then pipeline with the output DMAs.
  - Things measured SLOWER and reverted: deeper T pool (28/32 bufs),
    pre-emitting blk1 T-producers in the mid gap, moving tp/kt/v copies
    mostly to DVE, fused is_gt+mult stt reading zp PSUM directly
    (crashed HW: NRT_EXEC_UNIT_UNRECOVERABLE), xT split 3 ways with
    mqk first on sync.
"""

import sys

if "/opt/trn_rl_repo" not in sys.path:
    sys.path.insert(0, "/opt/trn_rl_repo")

import numpy as np
import ml_dtypes

import concourse.bass as bass  # noqa: F401
import concourse.tile as tile
from concourse import bacc, mybir
from concourse.bass_utils import run_bass_kernel_spmd

F32 = mybir.dt.float32
BF16 = mybir.dt.bfloat16
AL = mybir.AluOpType
AF = mybir.ActivationFunctionType

B, S, D = 2, 1024, 512
DH = D // 2          # 256 per-head dim
HID = 128            # predictor hidden
NCHUNK = S // 4      # 256 query rows per core
P = 128

def _mkpat(*ratios):
    """length-128 producer pattern from per-16 (v,s) ratios, cycled."""
    base = {
        (11, 5): ["v", "s", "v", "v", "s", "v", "v", "s",
                  "v", "v", "s", "v", "v", "s", "v", "v"],
        (10, 6): ["v", "s", "v", "s", "v", "v", "s", "v",
                  "s", "v", "v", "s", "v", "v", "s", "v"],
        (9, 7): ["v", "s", "v", "s", "v", "v", "s", "v",
                 "s", "v", "v", "s", "v", "s", "v", "s"],
    }
    out = []
    for g in range(8):
        out += base[ratios[g % len(ratios)]]
    return out


# knobs (test.py may override before first kernel() call)
CONFIG = {
    "trace": False,
    "tmpdir": None,
    # per-row producer pattern (length 128, indexed by i%128): 'v'=DVE
    # (744ns 1-op max), 's'=ACT (1136ns relu). 10:6 matches op costs;
    # blk1 leans ACT-ward since DVE also carries the mid softmax chain.
    # blk0 leans DVE-ward: ACT carries ~10us of pre-blk0 copies/exp
    "tpat": _mkpat((11, 5)),
    "tpat1": _mkpat((10, 6)),
    "t_bufs": 24,
}

_STATE = {}


def _emit(tc, nc, t):
    sl512 = [slice(0, 512), slice(512, 1024)]

    with tc.tile_pool(name="big", bufs=1) as big:
        # ---- persistent residents ----
        cf_s = big.tile([P, 68], F32, name="cf_s")             # b1|thr|wsel32|selv
        b1_s = cf_s[:, 0:1]
        thr_s = cf_s[:, 1:2]
        wsel_s = cf_s[:, 2:66]
        selv_s = cf_s[:, 66:68]   # 1.0 where block b's row i is DVE-produced
        thr2_s = big.tile([P, 2], F32, name="thr2_s")  # per-row mask threshold
        cb_s = big.tile([1, D + P], BF16, name="cb_s")          # bo | ones
        bo_s = cb_s[:, 0:D]
        one_s = cb_s[:, D : D + P]
        ident = big.tile([P, P], BF16, name="ident")

        at_s = big.tile([P, NCHUNK], F32, name="at_s")    # a^T + b1, [h, i]
        nat_s = big.tile([P, NCHUNK], F32, name="nat_s")  # -(a^T + b1)
        ct_s = big.tile([P, S], F32, name="ct_s")          # c^T, [h, j]

        xbt_s = big.tile([P, 4, S], BF16, name="xbt_s")       # x^T bf16
        wqb_s = big.tile([P, 4, D], BF16, name="wqb_s")       # Wq/16 bf16
        wkb_s = big.tile([P, 4, D], BF16, name="wkb_s")
        wvb_s = big.tile([P, 4, D], BF16, name="wvb_s")
        wob_s = big.tile([P, 4, D], BF16, name="wob_s")
        qt_s = big.tile([P, 4, NCHUNK], BF16, name="qt_s")  # q^T/16 [dout, i]
        kt_s = big.tile([P, 4, S], BF16, name="kt_s")       # k^T [dout, j]
        v_s = big.tile([P, 8, D], BF16, name="v_s")         # v [j(8 tiles), d]
        otr_s = big.tile([P, 4, NCHUNK], BF16, name="otr_s")  # out^T [d, i]

        with (
            tc.tile_pool(name="pjp", bufs=1, space="PSUM") as pjp,
            tc.tile_pool(name="zps", bufs=1, space="PSUM") as zpsp,
        ):
            # -------- stage A: DMAs + exact a/c (bf16 hi/lo splits) --------
            with tc.tile_pool(name="stageA", bufs=1) as sa:
                xlo_s = sa.tile([P, 4, S], BF16, name="xlo_s")   # x - bf16(x)
                mqh_s = sa.tile([P, 4, 2 * HID], BF16, name="mqh_s")
                mql_s = sa.tile([P, 4, 2 * HID], BF16, name="mql_s")
                # a/c are computed exactly via 3 bf16 streams
                # (Mhi@xhi + Mhi@xlo + Mlo@xhi; the dropped Mlo@xlo term is
                # ~2^-18 relative). Front-load x_hi/x_lo/M on the 2 HW
                # queues; weights ride gpsimd SWDGE (needed only in mid).
                # dt-half transfers keep 4KB/partition contiguous packets
                # (full queue rate) while letting the a/c matmuls start on
                # the first half. xb alone on scalar (feeds 2 of 3 streams)
                nc.scalar.dma_start(xbt_s[:, 0:2, :], t["xbT"][:, 0:2, :])
                nc.scalar.dma_start(xbt_s[:, 2:4, :], t["xbT"][:, 2:4, :])
                nc.sync.dma_start(cf_s[:], t["constsf"])
                nc.sync.dma_start(mqh_s[:], t["mqh"])
                nc.sync.dma_start(mql_s[:], t["mql"])
                nc.sync.dma_start(xlo_s[:, 0:2, :], t["xlo"][:, 0:2, :])
                nc.sync.dma_start(xlo_s[:, 2:4, :], t["xlo"][:, 2:4, :])
                nc.gpsimd.dma_start(wqb_s[:], t["wq"])
                nc.gpsimd.dma_start(wkb_s[:], t["wk"])
                nc.gpsimd.dma_start(cb_s[:], t["constsb"])
                nc.gpsimd.dma_start(ident[:], t["identb"])
                nc.gpsimd.dma_start(wvb_s[:], t["wv"])
                nc.gpsimd.dma_start(wob_s[:], t["wo"])

                # exact a/c: xb-based streams first (xb lands first), xlo
                # streams last; within each, dt-order tracks DMA halves.
                # ct j-half 0 first, then a, then ct j-half 1; at/nat and
                # the first T can then start while jc1 still computes.
                a_streams = [(mqh_s, xbt_s), (mql_s, xbt_s), (mqh_s, xlo_s)]

                ct_pss = []
                for hf in range(2):
                    ct_pss.append(pjp.tile([P, 512], F32, tag="vps", bufs=2, name="ct_ps"))

                def ct_mms(hf, streams, start=False, stop=False):
                    sl = sl512[hf]
                    ops = [(m_, x_, dt_) for m_, x_ in streams for dt_ in range(4)]
                    for n_, (m_, x_, dt_) in enumerate(ops):
                        nc.tensor.matmul(
                            ct_pss[hf][:], m_[:, dt_, HID : 2 * HID], x_[:, dt_, sl],
                            start=(start and n_ == 0), stop=(stop and n_ == len(ops) - 1),
                        )

                at_ps = zpsp.tile([P, NCHUNK], F32, tag="z", bufs=2, name="at_ps")
                ct_mms(0, a_streams[0:2], start=True)          # xb-based, jc0
                for st, (m_, x_) in enumerate(a_streams):      # a (all dt)
                    for dt_ in range(4):
                        nc.tensor.matmul(
                            at_ps[:], m_[:, dt_, 0:HID], x_[:, dt_, 0:NCHUNK],
                            start=(st == 0 and dt_ == 0),
                            stop=(st == 2 and dt_ == 3),
                        )
                ct_mms(0, a_streams[2:3], stop=True)           # xlo, jc0
                nc.vector.tensor_scalar(at_s[:], at_ps[:], b1_s[:], None, AL.add)
                nc.scalar.copy(ct_s[:, sl512[0]], ct_pss[0][:])
                ct_mms(1, a_streams, start=True, stop=True)    # jc1 (all)
                nc.vector.tensor_scalar(nat_s[:], at_s[:], -1.0, None, AL.mult)

                # alpha_b[i] = sum_h W2[h]*at[h, 128b+i]: DVE 'v' steps emit
                # T' = max(ct, -at) (1-op), so their z rows miss sum W2*at.
                # Fold into the mask threshold: thr2 = thr - alpha*selv.
                for b_ in range(2):
                    al_ps = zpsp.tile([P, 1], F32, tag="z", bufs=2, name="al_ps")
                    nc.tensor.matmul(
                        al_ps[:], at_s[:, P * b_ : P * (b_ + 1)],
                        wsel_s[:, 32:33], start=True, stop=True,
                    )
                    nc.vector.scalar_tensor_tensor(
                        thr2_s[:, b_ : b_ + 1], al_ps[:], -1.0,
                        selv_s[:, b_ : b_ + 1], AL.mult, AL.mult,
                    )
                    nc.vector.tensor_scalar(
                        thr2_s[:, b_ : b_ + 1], thr2_s[:, b_ : b_ + 1],
                        thr_s[:], None, AL.add,
                    )

                nc.scalar.copy(ct_s[:, sl512[1]], ct_pss[1][:])

            # ---------------- z blocks / mid / tail ------------------------
            with (
                tc.tile_pool(name="Tp", bufs=CONFIG["t_bufs"]) as Tp,
                tc.tile_pool(name="work", bufs=2) as work,
            ):
                att_sb = [
                    work.tile([P, 8, NCHUNK], BF16, tag="attnT", bufs=2,
                              name=f"a_sb{h}")
                    for h in range(2)
                ]
                # exp(scores) for all 4 (head, ti) pairs: mask-independent,
                # computed in the mid gap; masked in softmax_finish later.
                e_sb = [
                    [work.tile([P, S], BF16, tag="e", bufs=4, name=f"e{h}_{ti}")
                     for ti in range(2)]
                    for h in range(2)
                ]

                def make_T(ii):
                    T = Tp.tile([P, S], F32, tag="T", name=f"T{ii}")
                    pat = CONFIG["tpat"] if ii < 128 else CONFIG["tpat1"]
                    if pat[ii % 128] == "v":
                        # T' = max(ct, -a) (1-op): the missing +a_i is folded
                        # into the mask threshold via alpha (see thr2_s)
                        nc.vector.tensor_scalar(
                            T[:], ct_s[:], nat_s[:, ii : ii + 1], None, AL.max,
                        )
                    else:
                        nc.scalar.activation(
                            T[:], ct_s[:], AF.Relu, bias=at_s[:, ii : ii + 1]
                        )
                    return T

                def emit_zblock(blk, pre=()):
                    # z accumulation for 128 query rows; col groups MUST cycle
                    # (g=step%4): serial same-group fp32 mms run at 2cyc/col,
                    # cycling pipelines passes across the 4 array quadrants.
                    # Keep this stream PURE: a full-array matmul inserted here
                    # drains the quadrant pipeline (~1us each).
                    # `pre` carries T tiles already emitted during the mid gap.
                    zp = zpsp.tile([P, S], F32, tag="z", bufs=2, name=f"zp{blk}")
                    for step in range(128):
                        k, g = step // 4, step % 4
                        i = 32 * g + k
                        ii = blk * 128 + i
                        T = pre[step] if step < len(pre) else make_T(ii)
                        for jc in range(2):
                            nc.tensor.matmul(
                                zp[32 * g : 32 * g + 32, sl512[jc]],
                                wsel_s[:, 32 - k : 64 - k],
                                T[:, sl512[jc]],
                                start=(k == 0), stop=(k == 31),
                                tile_position=(0, 32 * g),
                                skip_group_check=True,
                            )
                    return zp

                def qkv_units():
                    # projections: dense bf16 PE chains (FWL weight loads),
                    # decomposed into units so blk1 T-producers interleave
                    us = []
                    def qt_u(dout):
                        def go():
                            qt_ps = pjp.tile([P, NCHUNK], F32, tag="vps", bufs=2, name="qt_ps")
                            for dt_ in range(4):
                                nc.tensor.matmul(
                                    qt_ps[:], wqb_s[:, dt_, 128 * dout : 128 * (dout + 1)],
                                    xbt_s[:, dt_, 0:NCHUNK], start=(dt_ == 0), stop=(dt_ == 3),
                                )
                            nc.scalar.copy(qt_s[:, dout, :], qt_ps[:])
                        return go
                    def kt_u(dout, jc):
                        def go():
                            kt_ps = pjp.tile([P, 512], F32, tag="vps", bufs=2, name="kt_ps")
                            for dt_ in range(4):
                                nc.tensor.matmul(
                                    kt_ps[:],
                                    wkb_s[:, dt_, 128 * dout : 128 * (dout + 1)],
                                    xbt_s[:, dt_, sl512[jc]],
                                    start=(dt_ == 0), stop=(dt_ == 3),
                                )
                            nc.scalar.copy(kt_s[:, dout, sl512[jc]], kt_ps[:])
                        return go
                    def v_u(jt):
                        def go():
                            v_ps = pjp.tile([P, D], F32, tag="vps", bufs=2, name="v_ps")
                            for dt_ in range(4):
                                nc.tensor.matmul(
                                    v_ps[:], xbt_s[:, dt_, 128 * jt : 128 * (jt + 1)],
                                    wvb_s[:, dt_, :], start=(dt_ == 0), stop=(dt_ == 3),
                                )
                            nc.scalar.copy(v_s[:, jt, :], v_ps[:])
                        return go
                    us += [qt_u(d) for d in range(4)]
                    us += [kt_u(d, jc) for d in range(4) for jc in range(2)]
                    return us, [v_u(jt) for jt in range(8)]

                def emit_scores(h, ti):
                    # scores + exp for rows [128*ti,128*(ti+1)) of head h.
                    # Mask-independent: runs in the mid gap for both ti.
                    for jc in range(2):
                        scp = pjp.tile([P, 512], F32, tag="vps", bufs=2,
                                       name=f"sc{h}{ti}{jc}")
                        for dt_ in range(2):
                            nc.tensor.matmul(
                                scp[:],
                                qt_s[:, 2 * h + dt_, 128 * ti : 128 * (ti + 1)],
                                kt_s[:, 2 * h + dt_, sl512[jc]],
                                start=(dt_ == 0), stop=(dt_ == 1),
                            )
                        # scores/16 bounded (|sc|<~7): exp cannot overflow,
                        # rowmax subtraction dropped (identical result)
                        nc.scalar.activation(
                            e_sb[h][ti][:, sl512[jc]], scp[:], AF.Exp
                        )

                em_sb = [[None, None], [None, None]]

                mask_sb = [None, None]

                def emit_mask(blk, zp):
                    m = work.tile([P, S], BF16, tag="mask", bufs=2, name=f"mask{blk}")
                    mask_sb[blk] = m
                    nc.vector.tensor_scalar(
                        m[:], zp[:], thr2_s[:, blk : blk + 1], None, AL.is_gt
                    )

                def softmax_finish(h, ti, zp):
                    # em = (e*mask + ind)/(sum + 1024*ind)
                    e = e_sb[h][ti]
                    em = work.tile([P, S], BF16, tag="em", bufs=4, name=f"em{h}{ti}")
                    em_sb[h][ti] = em
                    ssum = work.tile([P, 1], F32, tag="ssum", name="ssum")
                    nc.vector.scalar_tensor_tensor(
                        em[:], e[:], 0.0, mask_sb[ti][:], AL.add, AL.mult,
                        accum_out=ssum[:],
                    )
                    # fully-masked rows: reference = uniform 1/1024.
                    ind = work.tile([P, 1], F32, tag="ind", name="ind")
                    nc.vector.tensor_scalar(ind[:], ssum[:], 0.0, None, AL.is_equal)
                    s2 = work.tile([P, 1], F32, tag="s2", name="s2")
                    nc.vector.tensor_scalar(s2[:], ind[:], 1024.0, ssum[:], AL.mult, AL.add)
                    rinv = work.tile([P, 1], F32, tag="rinv", name="rinv")
                    nc.vector.reciprocal(rinv[:], s2[:])
                    nc.vector.tensor_scalar(em[:], em[:], ind[:], rinv[:], AL.add, AL.mult)

                def emit_transposes(h, ti, eng="s"):
                    # mid copies on ACT (the DVE queue is the blk1
                    # pace-setter); tail copies on V (ACT carries the v
                    # copies there). 4 transposes batch into one PSUM tile
                    # -> one [P,4,128] copy (fewer, fatter copies; no
                    # transpose/copy ping-pong on the 2 bufs).
                    em = em_sb[h][ti]
                    for g in range(2):
                        tp_ps = pjp.tile([P, 4, P], BF16, tag="tp", bufs=2, name="tp_ps")
                        for j2 in range(4):
                            jt = 4 * g + j2
                            nc.tensor.transpose(
                                tp_ps[:, j2, :], em[:, 128 * jt : 128 * (jt + 1)],
                                ident[:],
                            )
                        dst = att_sb[h][:, 4 * g : 4 * (g + 1), 128 * ti : 128 * (ti + 1)]
                        if eng == "s":
                            nc.scalar.copy(dst, tp_ps[:])
                        else:
                            nc.vector.tensor_copy(dst, tp_ps[:])

                def emit_av(h, ti):
                    # ti-split: the ti=0 half depends only on mid-gap
                    # transposes, so it runs in the mid; tail does ti=1 only
                    isl = slice(128 * ti, 128 * (ti + 1))
                    for dt_ in range(2):
                        ot_ps = pjp.tile([P, P], F32, tag="vps", bufs=2, name="ot_ps")
                        for jt in range(8):
                            nc.tensor.matmul(
                                ot_ps[:],
                                v_s[:, jt, 256 * h + 128 * dt_ : 256 * h + 128 * (dt_ + 1)],
                                att_sb[h][:, jt, isl],
                                start=(jt == 0), stop=(jt == 7),
                            )
                        if (dt_ + ti) % 2 == 0:
                            nc.vector.tensor_copy(otr_s[:, 2 * h + dt_, isl], ot_ps[:])
                        else:
                            nc.scalar.copy(otr_s[:, 2 * h + dt_, isl], ot_ps[:])

                def emit_oproj(ti):
                    # out rows [128ti,128ti+128): bias + 4 otr-weighted MMs,
                    # then SBUF copy + the output DMA for that half
                    o_ps = zpsp.tile([P, D], F32, tag="z", bufs=2, name=f"o_ps{ti}")
                    nc.tensor.matmul(o_ps[:], one_s[:], bo_s[:], start=True, stop=False)
                    for dt_ in range(4):
                        nc.tensor.matmul(
                            o_ps[:], otr_s[:, dt_, 128 * ti : 128 * (ti + 1)],
                            wob_s[:, dt_, :], start=False, stop=(dt_ == 3),
                        )
                    o_sb = work.tile([P, D], F32, tag="osb", bufs=2, name="o_sb")
                    if ti == 0:
                        nc.scalar.copy(o_sb[:], o_ps[:])
                    else:
                        nc.vector.tensor_copy(o_sb[:], o_ps[:])
                    nc.sync.dma_start(t["out"][128 * ti : 128 * (ti + 1), :], o_sb[:])

                # ---- emission schedule ----
                # setup (pre-blk0): QKV + scores + exp. All their PSUM
                # copies ride ACT, so V's T-production starts as soon as
                # ct/at land; PE is DMA-gated here anyway. Head-ordered so
                # sc(h) follows its 6 prerequisite units.
                qk_us, v_us = qkv_units()
                for u in (qk_us[0], qk_us[1], qk_us[4], qk_us[5], qk_us[6], qk_us[7]):
                    u()
                emit_scores(0, 0)
                emit_scores(0, 1)
                for u in (qk_us[2], qk_us[3], qk_us[8], qk_us[9], qk_us[10], qk_us[11]):
                    u()
                emit_scores(1, 0)
                emit_scores(1, 1)

                zp0 = emit_zblock(0)

                # mid gap: ONLY the zp0 chain -- V: mask + both ti=0
                # softmaxes (contiguous, ahead of pool-stalled preTs);
                # ACT: the tp(0,0) copies; then preTs refill the blk1 pool.
                pre = []
                def preT(n):
                    for _ in range(n):
                        step = len(pre)
                        k, g = step // 4, step % 4
                        pre.append(make_T(128 + 32 * g + k))
                emit_mask(0, zp0)
                softmax_finish(0, 0, zp0)
                softmax_finish(1, 0, zp0)
                emit_transposes(0, 0, "s")
                preT(18)

                zp1 = emit_zblock(1, pre)

                # tail: V runs mask1 + both ti=1 softmaxes then tp(.,1)
                # copies; ACT takes tp(1,0) (deps mid-done: first, ahead of
                # the v copies) then v; PE fills with the deferred V
                # projection + ti=0 attention half + out rows 0:128 while
                # the ti=1 chain lands last.
                emit_mask(1, zp1)
                softmax_finish(0, 1, zp1)
                softmax_finish(1, 1, zp1)
                emit_transposes(1, 0, "s")
                for u in v_us:
                    u()
                emit_transposes(0, 1, "v")
                emit_transposes(1, 1, "v")
                emit_av(0, 0)
                emit_av(1, 0)
                emit_oproj(0)
                emit_av(0, 1)
                emit_av(1, 1)
                emit_oproj(1)


def _build():
    if "nc" in _STATE:
        return _STATE["nc"]
    nc = bacc.Bacc(
        "TRN2", target_bir_lowering=False, debug=False, enable_asserts=True,
        num_devices=8,
    )
    t = {}
    t["xbT"] = nc.dram_tensor("xbT", [P, 4, S], BF16, kind="ExternalInput").ap()
    t["xlo"] = nc.dram_tensor("xlo", [P, 4, S], BF16, kind="ExternalInput").ap()
    t["wq"] = nc.dram_tensor("wq", [P, 4, D], BF16, kind="ExternalInput").ap()
    t["wk"] = nc.dram_tensor("wk", [P, 4, D], BF16, kind="ExternalInput").ap()
    t["wv"] = nc.dram_tensor("wv", [P, 4, D], BF16, kind="ExternalInput").ap()
    t["wo"] = nc.dram_tensor("wo", [P, 4, D], BF16, kind="ExternalInput").ap()
    t["mqh"] = nc.dram_tensor("mqh", [P, 4, 2 * HID], BF16, kind="ExternalInput").ap()
    t["mql"] = nc.dram_tensor("mql", [P, 4, 2 * HID], BF16, kind="ExternalInput").ap()
    t["constsf"] = nc.dram_tensor("constsf", [P, 68], F32, kind="ExternalInput").ap()
    t["constsb"] = nc.dram_tensor("constsb", [1, D + P], BF16, kind="ExternalInput").ap()
    t["identb"] = nc.dram_tensor("identb", [P, P], BF16, kind="ExternalInput").ap()
    t["out"] = nc.dram_tensor("out", [NCHUNK, D], F32, kind="ExternalOutput").ap()

    with tile.TileContext(nc) as tc:
        _emit(tc, nc, t)
    nc.compile()
    _STATE["nc"] = nc
    return nc


def _prep_in_maps(inputs):
    bf16 = ml_dtypes.bfloat16
    x = np.ascontiguousarray(np.asarray(inputs["x"], np.float32))
    Wq = np.asarray(inputs["Wq"], np.float32)
    Wk = np.asarray(inputs["Wk"], np.float32)
    Wv = np.asarray(inputs["Wv"], np.float32)
    Wo = np.asarray(inputs["Wo"], np.float32)
    bo = np.asarray(inputs["bo"], np.float32)
    W1 = np.asarray(inputs["W1"], np.float64)
    b1 = np.asarray(inputs["b1"], np.float32)
    W2 = np.asarray(inputs["W2"], np.float32)
    b2 = np.asarray(inputs["b2"], np.float32)

    wq_m = 0.5 * (Wq[:, :DH].astype(np.float64) + Wq[:, DH:].astype(np.float64))
    wk_m = 0.5 * (Wk[:, :DH].astype(np.float64) + Wk[:, DH:].astype(np.float64))
    Mq = np.ascontiguousarray((wq_m @ W1[:DH]).astype(np.float32))
    Mk = np.ascontiguousarray((wk_m @ W1[DH:]).astype(np.float32))

    def chunk(a):
        # [D, N] -> [P, 4, N]: partition-chunked layout for one-shot DMA
        return np.ascontiguousarray(a.reshape(4, P, -1).transpose(1, 0, 2))

    constsf = np.zeros((P, 68), np.float32)
    constsf[:, 0] = b1
    constsf[:, 1] = -float(b2[0])
    constsf[:, 2 + 32] = W2[:, 0]          # wsel32 window buffer
    # selv[b][i]=1 if block b's query row i is produced by the DVE 1-op
    # max path (z row then needs the alpha threshold correction)
    for b_, pat in ((0, CONFIG["tpat"]), (1, CONFIG["tpat1"])):
        constsf[:, 66 + b_] = np.array(
            [1.0 if pat[i % 128] == "v" else 0.0 for i in range(P)], np.float32
        )
    constsb = np.zeros((1, D + P), bf16)
    constsb[0, :D] = bo.astype(bf16)
    constsb[0, D:] = np.ones(P, bf16)

    M = np.concatenate([Mq, Mk], axis=1)
    Mh = M.astype(bf16)
    shared = dict(
        wq=chunk((Wq / 16.0).astype(bf16)),
        wk=chunk(Wk.astype(bf16)),
        wv=chunk(Wv.astype(bf16)),
        wo=chunk(Wo.astype(bf16)),
        mqh=chunk(Mh),
        mql=chunk((M - Mh.astype(np.float32)).astype(bf16)),
        constsf=constsf, constsb=constsb,
        identb=np.eye(P, dtype=bf16),
    )
    in_maps = []
    for c in range(8):
        b, i0 = c // 4, (c % 4) * NCHUNK
        m = dict(shared)
        # roll x columns so this core's query chunk sits at j=0..255; the
        # softmax result is invariant to a consistent j-permutation of
        # keys/values/mask, and it lets `a` start from the first x DMA.
        xr = np.roll(x[b].T, -i0, axis=1)
        xh = xr.astype(bf16)
        m["xbT"] = chunk(xh)
        m["xlo"] = chunk((xr - xh.astype(np.float32)).astype(bf16))
        in_maps.append(m)
    return in_maps


def kernel(**inputs):
    nc = _build()
    in_maps = _prep_in_maps(inputs)
    res = run_bass_kernel_spmd(
        nc, in_maps, core_ids=list(range(8)),
        trace=CONFIG["trace"], tmpdir=CONFIG["tmpdir"],
    )
    _STATE["last_result"] = res
    out = np.empty((B, S, D), np.float32)
    for c in range(8):
        b, i0 = c // 4, (c % 4) * NCHUNK
        out[b, i0 : i0 + NCHUNK] = res.results[c]["out"]
    return out



# revision 49
# speedup vs baseline: 1.0524x; 1.0524x over previous
"""AdaptiveSparsityAttention TRN2 kernel (8 NeuronCores, SPMD data-parallel).

Problem (B=2, S=1024, D=512, H=2 heads, dh=256, hidden=128):
  q,k,v = x@Wq, x@Wk, x@Wv (split 2 heads); scores = q@k^T/16
  a_i = q_mean@W1[:dh]+b1, c_j = k_mean@W1[dh:]
  z[i,j] = W2 . relu(a_i + c_j)          (sigmoid(z+b2)>0.5  <=>  z > -b2)
  attn = softmax(mask(scores));  out = (attn@v)@Wo + bo

Sharding: 8 cores = 2 batches x 4 query-chunks of 256 rows. Each core
computes its output chunk fully locally, no collectives.

Precision structure (measured, numpy sims):
  - z MUST be fp32-exact: z std 0.022, ~46% fill, threshold 0. bf16-level
    z error flips ~0.1% of mask bits -> 5.8e-2 L2 (FAILS 2e-2 gate);
    fp16 -> 1.5e-2 (too marginal). fp32 mask matches jax ref at 1.4e-3.
  - Everything downstream of the mask is bf16-safe: bf16 q/k/v/scores/
    attn-weights/out-proj measured 4.1e-3 total vs jax ref (5x margin).
    bf16 (vs baseline's float32r) buys FWL fast weight loads on PE,
    2-4x DVE modes, and half the DMA bytes.

Performance structure (per core, from perfetto traces; 194-201us HW
at full clock (best 194823), vs 217us for the float32r baseline; the chip sometimes
drops to 2.0GHz P0 throttle under sustained benching -- all engine
durations then scale ~1.2x, check zMM med dur 592 vs 710):
  - z stream: 2 blocks x 128 steps (1 query row each) of [T-tile produce
    on DVE (~805ns) or ACT (~1136ns)] + 2 fp32 matmul calls (wsel delta
    trick, 4-way col-tiled, 4 LDW + 4 MM instrs/step, LDW ~110ns each).
    Pace ~470ns/step in blk0, T-production + LDW co-bound. Near the
    structural floor: fp32 MMs cannot share LDWs in bass (explicit
    ldweights is broken for fp32/f32r), and exact T production is
    elementwise fp32 on DVE/ACT (tensor_scalar = 2 elem/cyc/lane).
  - The z streams are kept PURE: interleaving full-array matmuls into
    the col-tiled fp32 stream drains the quadrant pipeline (~1us each;
    measured +23us/block). QKV/scores/exp/blk0-softmax run in the mid
    gap; only zp1-dependent softmax + AV + out-proj are in the tail.
  - x is column-rolled per core (host side) so the query chunk is
    j=0..255: the fp32 a-matmuls start from the first x DMA quarter and
    no separate xq transfer is needed. Softmax is invariant to the
    consistent j-permutation of keys/values/mask.
  - One DMA instruction per tensor (the DGE costs ~600ns/descriptor;
    actual transfer bandwidth ~150GB/s/queue, shared DRAM channel), on
    the 2 HW queues (sync/scalar) + gpsimd SWDGE for late-needed data.
  - The V projection + its copies and head1's ti=0 transposes are
    deferred past blk1 (first needed by AV in the tail, where PE idles
    during zp1's softmax): -9us off blk1 by shrinking the mid-gap
    DVE/ACT spill that T-starves it. Deferring the WHOLE ti=0 attention
    chain went too far: blk1 hit 69us but the tail blew up +11.5
    (the chain serializes; blk1+tail work is ~conserved, ~108us).
    12 of blk1's T-producers are pre-emitted among the mid qk units
    (body pace is LDW-bound at 429 vs 456ns/step production, so the
    pool drains only ~27ns/step; thinner interleave unblocks the mid's
    exp/copy deps sooner). The mid is ordered per-head (head h's scores
    need only qt/kt dout 2h..2h+1, emitted right after those 6 units):
    halving the sc matmuls' dependency depth cut blk1's ramp-in stall,
    74.9->69.3us. Moving head1's ti=0 softmax to the tail instead was
    a pure regression (+2.7 tail, ramp unchanged -- the ramp binder is
    ACT-queue exp/relu ordering, not the DVE softmax). The out-proj is
    split around av(1): bias+head0 accumulation fills the PE stall
    while tp(1,1) copies drain, reusing the dead zp PSUM slots; both
    o_sb copies then pipeline with the output DMAs.
  - Things measured SLOWER and reverted: deeper T pool (28/32 bufs),
    pre-emitting blk1 T-producers in the mid gap, moving tp/kt/v copies
    mostly to DVE, fused is_gt+mult stt reading zp PSUM directly
    (crashed HW: NRT_EXEC_UNIT_UNRECOVERABLE), xT split 3 ways with
    mqk first on sync.
"""

import sys

if "/opt/trn_rl_repo" not in sys.path:
    sys.path.insert(0, "/opt/trn_rl_repo")

import numpy as np
import ml_dtypes

import concourse.bass as bass  # noqa: F401
import concourse.tile as tile
from concourse import bacc, mybir
from concourse.bass_utils import run_bass_kernel_spmd

F32 = mybir.dt.float32
BF16 = mybir.dt.bfloat16
AL = mybir.AluOpType
AF = mybir.ActivationFunctionType

B, S, D = 2, 1024, 512
DH = D // 2          # 256 per-head dim
HID = 128            # predictor hidden
NCHUNK = S // 4      # 256 query rows per core
P = 128

def _mkpat(*ratios):
    """length-128 producer pattern from per-16 (v,s) ratios, cycled."""
    base = {
        (10, 6): ["v", "s", "v", "s", "v", "v", "s", "v",
                  "s", "v", "v", "s", "v", "v", "s", "v"],
        (9, 7): ["v", "s", "v", "s", "v", "v", "s", "v",
                 "s", "v", "v", "s", "v", "s", "v", "s"],
    }
    out = []
    for g in range(8):
        out += base[ratios[g % len(ratios)]]
    return out


# knobs (test.py may override before first kernel() call)
CONFIG = {
    "trace": False,
    "tmpdir": None,
    # per-row producer pattern (length 128, indexed by i%128): 'v'=DVE
    # (744ns 1-op max), 's'=ACT (1136ns relu). 10:6 matches op costs;
    # blk1 leans ACT-ward since DVE also carries the mid softmax chain.
    "tpat": _mkpat((10, 6)),
    "tpat1": _mkpat((10, 6)),
    "t_bufs": 24,
}

_STATE = {}


def _emit(tc, nc, t):
    sl512 = [slice(0, 512), slice(512, 1024)]

    with tc.tile_pool(name="big", bufs=1) as big:
        # ---- persistent residents ----
        cf_s = big.tile([P, 68], F32, name="cf_s")             # b1|thr|wsel32|selv
        b1_s = cf_s[:, 0:1]
        thr_s = cf_s[:, 1:2]
        wsel_s = cf_s[:, 2:66]
        selv_s = cf_s[:, 66:68]   # 1.0 where block b's row i is DVE-produced
        thr2_s = big.tile([P, 2], F32, name="thr2_s")  # per-row mask threshold
        cb_s = big.tile([1, D + P], BF16, name="cb_s")          # bo | ones
        bo_s = cb_s[:, 0:D]
        one_s = cb_s[:, D : D + P]
        ident = big.tile([P, P], BF16, name="ident")

        at_s = big.tile([P, NCHUNK], F32, name="at_s")    # a^T + b1, [h, i]
        nat_s = big.tile([P, NCHUNK], F32, name="nat_s")  # -(a^T + b1)
        ct_s = big.tile([P, S], F32, name="ct_s")          # c^T, [h, j]

        xbt_s = big.tile([P, 4, S], BF16, name="xbt_s")       # x^T bf16
        wqb_s = big.tile([P, 4, D], BF16, name="wqb_s")       # Wq/16 bf16
        wkb_s = big.tile([P, 4, D], BF16, name="wkb_s")
        wvb_s = big.tile([P, 4, D], BF16, name="wvb_s")
        wob_s = big.tile([P, 4, D], BF16, name="wob_s")
        qt_s = big.tile([P, 4, NCHUNK], BF16, name="qt_s")  # q^T/16 [dout, i]
        kt_s = big.tile([P, 4, S], BF16, name="kt_s")       # k^T [dout, j]
        v_s = big.tile([P, 8, D], BF16, name="v_s")         # v [j(8 tiles), d]
        otr_s = big.tile([P, 4, NCHUNK], BF16, name="otr_s")  # out^T [d, i]

        with (
            tc.tile_pool(name="pjp", bufs=1, space="PSUM") as pjp,
            tc.tile_pool(name="zps", bufs=1, space="PSUM") as zpsp,
        ):
            # -------- stage A: DMAs + exact a/c (bf16 hi/lo splits) --------
            with tc.tile_pool(name="stageA", bufs=1) as sa:
                xlo_s = sa.tile([P, 4, S], BF16, name="xlo_s")   # x - bf16(x)
                mqh_s = sa.tile([P, 4, 2 * HID], BF16, name="mqh_s")
                mql_s = sa.tile([P, 4, 2 * HID], BF16, name="mql_s")
                # a/c are computed exactly via 3 bf16 streams
                # (Mhi@xhi + Mhi@xlo + Mlo@xhi; the dropped Mlo@xlo term is
                # ~2^-18 relative). Front-load x_hi/x_lo/M on the 2 HW
                # queues; weights ride gpsimd SWDGE (needed only in mid).
                # dt-half transfers keep 4KB/partition contiguous packets
                # (full queue rate) while letting the a/c matmuls start on
                # the first half. xb alone on scalar (feeds 2 of 3 streams)
                nc.scalar.dma_start(xbt_s[:, 0:2, :], t["xbT"][:, 0:2, :])
                nc.scalar.dma_start(xbt_s[:, 2:4, :], t["xbT"][:, 2:4, :])
                nc.sync.dma_start(cf_s[:], t["constsf"])
                nc.sync.dma_start(mqh_s[:], t["mqh"])
                nc.sync.dma_start(mql_s[:], t["mql"])
                nc.sync.dma_start(xlo_s[:, 0:2, :], t["xlo"][:, 0:2, :])
                nc.sync.dma_start(xlo_s[:, 2:4, :], t["xlo"][:, 2:4, :])
                nc.gpsimd.dma_start(wqb_s[:], t["wq"])
                nc.gpsimd.dma_start(wkb_s[:], t["wk"])
                nc.gpsimd.dma_start(cb_s[:], t["constsb"])
                nc.gpsimd.dma_start(ident[:], t["identb"])
                nc.gpsimd.dma_start(wvb_s[:], t["wv"])
                nc.gpsimd.dma_start(wob_s[:], t["wo"])

                # exact a/c: xb-based streams first (xb lands first), the
                # xlo stream last; ct j-half 0 first, then a, then j-half 1
                # so at/nat + the jc0 copy land while jc1 still computes.
                a_streams = [(mqh_s, xbt_s), (mql_s, xbt_s), (mqh_s, xlo_s)]
                ct_pss = [
                    pjp.tile([P, 512], F32, tag="vps", bufs=2, name="ct_ps")
                    for _ in range(2)
                ]

                def ct_mms(hf, streams, start=False, stop=False):
                    ops = [(m_, x_, dt_) for m_, x_ in streams for dt_ in range(4)]
                    for n_, (m_, x_, dt_) in enumerate(ops):
                        nc.tensor.matmul(
                            ct_pss[hf][:], m_[:, dt_, HID : 2 * HID],
                            x_[:, dt_, sl512[hf]],
                            start=(start and n_ == 0),
                            stop=(stop and n_ == len(ops) - 1),
                        )

                at_ps = zpsp.tile([P, NCHUNK], F32, tag="z", bufs=2, name="at_ps")
                ct_mms(0, a_streams[0:2], start=True)          # xb-based, jc0
                for st, (m_, x_) in enumerate(a_streams):      # a (all dt)
                    for dt_ in range(4):
                        nc.tensor.matmul(
                            at_ps[:], m_[:, dt_, 0:HID], x_[:, dt_, 0:NCHUNK],
                            start=(st == 0 and dt_ == 0),
                            stop=(st == 2 and dt_ == 3),
                        )
                ct_mms(0, a_streams[2:3], stop=True)           # xlo, jc0
                nc.vector.tensor_scalar(at_s[:], at_ps[:], b1_s[:], None, AL.add)
                nc.scalar.copy(ct_s[:, sl512[0]], ct_pss[0][:])
                ct_mms(1, a_streams, start=True, stop=True)    # jc1 (all)
                nc.vector.tensor_scalar(nat_s[:], at_s[:], -1.0, None, AL.mult)

                # alpha_b[i] = sum_h W2[h]*at[h, 128b+i]: DVE 'v' steps emit
                # T' = max(ct, -at) (1-op), so their z rows miss sum W2*at.
                # Fold into the mask threshold: thr2 = thr - alpha*selv.
                for b_ in range(2):
                    al_ps = zpsp.tile([P, 1], F32, tag="z", bufs=2, name="al_ps")
                    nc.tensor.matmul(
                        al_ps[:], at_s[:, P * b_ : P * (b_ + 1)],
                        wsel_s[:, 32:33], start=True, stop=True,
                    )
                    nc.vector.scalar_tensor_tensor(
                        thr2_s[:, b_ : b_ + 1], al_ps[:], -1.0,
                        selv_s[:, b_ : b_ + 1], AL.mult, AL.mult,
                    )
                    nc.vector.tensor_scalar(
                        thr2_s[:, b_ : b_ + 1], thr2_s[:, b_ : b_ + 1],
                        thr_s[:], None, AL.add,
                    )

                nc.scalar.copy(ct_s[:, sl512[1]], ct_pss[1][:])

            # ---------------- z blocks / mid / tail ------------------------
            with (
                tc.tile_pool(name="Tp", bufs=CONFIG["t_bufs"]) as Tp,
                tc.tile_pool(name="work", bufs=2) as work,
            ):
                att_sb = [
                    work.tile([P, 8, NCHUNK], BF16, tag="attnT", bufs=2,
                              name=f"a_sb{h}")
                    for h in range(2)
                ]
                # exp(scores) for all 4 (head, ti) pairs: mask-independent,
                # computed in the mid gap; masked in softmax_finish later.
                e_sb = [
                    [work.tile([P, S], BF16, tag="e", bufs=4, name=f"e{h}_{ti}")
                     for ti in range(2)]
                    for h in range(2)
                ]

                def make_T(ii):
                    T = Tp.tile([P, S], F32, tag="T", name=f"T{ii}")
                    pat = CONFIG["tpat"] if ii < 128 else CONFIG["tpat1"]
                    if pat[ii % 128] == "v":
                        # T' = max(ct, -a) (1-op): the missing +a_i is folded
                        # into the mask threshold via alpha (see thr2_s)
                        nc.vector.tensor_scalar(
                            T[:], ct_s[:], nat_s[:, ii : ii + 1], None, AL.max,
                        )
                    else:
                        nc.scalar.activation(
                            T[:], ct_s[:], AF.Relu, bias=at_s[:, ii : ii + 1]
                        )
                    return T

                def emit_zblock(blk, pre=()):
                    # z accumulation for 128 query rows; col groups MUST cycle
                    # (g=step%4): serial same-group fp32 mms run at 2cyc/col,
                    # cycling pipelines passes across the 4 array quadrants.
                    # Keep this stream PURE: a full-array matmul inserted here
                    # drains the quadrant pipeline (~1us each).
                    # `pre` carries T tiles already emitted during the mid gap.
                    zp = zpsp.tile([P, S], F32, tag="z", bufs=2, name=f"zp{blk}")
                    for step in range(128):
                        k, g = step // 4, step % 4
                        i = 32 * g + k
                        ii = blk * 128 + i
                        T = pre[step] if step < len(pre) else make_T(ii)
                        for jc in range(2):
                            nc.tensor.matmul(
                                zp[32 * g : 32 * g + 32, sl512[jc]],
                                wsel_s[:, 32 - k : 64 - k],
                                T[:, sl512[jc]],
                                start=(k == 0), stop=(k == 31),
                                tile_position=(0, 32 * g),
                                skip_group_check=True,
                            )
                    return zp

                def qkv_units():
                    # projections: dense bf16 PE chains (FWL weight loads),
                    # decomposed into units so blk1 T-producers interleave
                    us = []
                    def qt_u(dout):
                        def go():
                            qt_ps = pjp.tile([P, NCHUNK], F32, tag="vps", bufs=2, name="qt_ps")
                            for dt_ in range(4):
                                nc.tensor.matmul(
                                    qt_ps[:], wqb_s[:, dt_, 128 * dout : 128 * (dout + 1)],
                                    xbt_s[:, dt_, 0:NCHUNK], start=(dt_ == 0), stop=(dt_ == 3),
                                )
                            nc.scalar.copy(qt_s[:, dout, :], qt_ps[:])
                        return go
                    def kt_u(dout, jc):
                        def go():
                            kt_ps = pjp.tile([P, 512], F32, tag="vps", bufs=2, name="kt_ps")
                            for dt_ in range(4):
                                nc.tensor.matmul(
                                    kt_ps[:],
                                    wkb_s[:, dt_, 128 * dout : 128 * (dout + 1)],
                                    xbt_s[:, dt_, sl512[jc]],
                                    start=(dt_ == 0), stop=(dt_ == 3),
                                )
                            nc.scalar.copy(kt_s[:, dout, sl512[jc]], kt_ps[:])
                        return go
                    def v_u(jt):
                        def go():
                            v_ps = pjp.tile([P, D], F32, tag="vps", bufs=2, name="v_ps")
                            for dt_ in range(4):
                                nc.tensor.matmul(
                                    v_ps[:], xbt_s[:, dt_, 128 * jt : 128 * (jt + 1)],
                                    wvb_s[:, dt_, :], start=(dt_ == 0), stop=(dt_ == 3),
                                )
                            nc.scalar.copy(v_s[:, jt, :], v_ps[:])
                        return go
                    us += [qt_u(d) for d in range(4)]
                    us += [kt_u(d, jc) for d in range(4) for jc in range(2)]
                    return us, [v_u(jt) for jt in range(8)]

                def emit_scores(h, ti):
                    # scores + exp for rows [128*ti,128*(ti+1)) of head h.
                    # Mask-independent: runs in the mid gap for both ti.
                    for jc in range(2):
                        scp = pjp.tile([P, 512], F32, tag="vps", bufs=2,
                                       name=f"sc{h}{ti}{jc}")
                        for dt_ in range(2):
                            nc.tensor.matmul(
                                scp[:],
                                qt_s[:, 2 * h + dt_, 128 * ti : 128 * (ti + 1)],
                                kt_s[:, 2 * h + dt_, sl512[jc]],
                                start=(dt_ == 0), stop=(dt_ == 1),
                            )
                        # scores/16 bounded (|sc|<~7): exp cannot overflow,
                        # rowmax subtraction dropped (identical result)
                        nc.scalar.activation(
                            e_sb[h][ti][:, sl512[jc]], scp[:], AF.Exp
                        )

                em_sb = [[None, None], [None, None]]

                mask_sb = [None, None]

                def emit_mask(blk, zp):
                    m = work.tile([P, S], BF16, tag="mask", bufs=2, name=f"mask{blk}")
                    mask_sb[blk] = m
                    nc.vector.tensor_scalar(
                        m[:], zp[:], thr2_s[:, blk : blk + 1], None, AL.is_gt
                    )

                def softmax_finish(h, ti, zp):
                    # em = (e*mask + ind)/(sum + 1024*ind)
                    e = e_sb[h][ti]
                    em = work.tile([P, S], BF16, tag="em", bufs=4, name=f"em{h}{ti}")
                    em_sb[h][ti] = em
                    ssum = work.tile([P, 1], F32, tag="ssum", name="ssum")
                    nc.vector.scalar_tensor_tensor(
                        em[:], e[:], 0.0, mask_sb[ti][:], AL.add, AL.mult,
                        accum_out=ssum[:],
                    )
                    # fully-masked rows: reference = uniform 1/1024.
                    ind = work.tile([P, 1], F32, tag="ind", name="ind")
                    nc.vector.tensor_scalar(ind[:], ssum[:], 0.0, None, AL.is_equal)
                    s2 = work.tile([P, 1], F32, tag="s2", name="s2")
                    nc.vector.tensor_scalar(s2[:], ind[:], 1024.0, ssum[:], AL.mult, AL.add)
                    rinv = work.tile([P, 1], F32, tag="rinv", name="rinv")
                    nc.vector.reciprocal(rinv[:], s2[:])
                    nc.vector.tensor_scalar(em[:], em[:], ind[:], rinv[:], AL.add, AL.mult)

                def emit_transposes(h, ti, eng="s"):
                    # mid copies on ACT (the DVE queue is the blk1
                    # pace-setter); tail copies on V (ACT carries the v
                    # copies there). 4 transposes batch into one PSUM tile
                    # -> one [P,4,128] copy (fewer, fatter copies; no
                    # transpose/copy ping-pong on the 2 bufs).
                    em = em_sb[h][ti]
                    for g in range(2):
                        tp_ps = pjp.tile([P, 4, P], BF16, tag="tp", bufs=2, name="tp_ps")
                        for j2 in range(4):
                            jt = 4 * g + j2
                            nc.tensor.transpose(
                                tp_ps[:, j2, :], em[:, 128 * jt : 128 * (jt + 1)],
                                ident[:],
                            )
                        dst = att_sb[h][:, 4 * g : 4 * (g + 1), 128 * ti : 128 * (ti + 1)]
                        if eng == "s":
                            nc.scalar.copy(dst, tp_ps[:])
                        else:
                            nc.vector.tensor_copy(dst, tp_ps[:])

                def emit_av(h, ti):
                    # ti-split: the ti=0 half depends only on mid-gap
                    # transposes, so it runs in the mid; tail does ti=1 only
                    isl = slice(128 * ti, 128 * (ti + 1))
                    for dt_ in range(2):
                        ot_ps = pjp.tile([P, P], F32, tag="vps", bufs=2, name="ot_ps")
                        for jt in range(8):
                            nc.tensor.matmul(
                                ot_ps[:],
                                v_s[:, jt, 256 * h + 128 * dt_ : 256 * h + 128 * (dt_ + 1)],
                                att_sb[h][:, jt, isl],
                                start=(jt == 0), stop=(jt == 7),
                            )
                        if (dt_ + ti) % 2 == 0:
                            nc.vector.tensor_copy(otr_s[:, 2 * h + dt_, isl], ot_ps[:])
                        else:
                            nc.scalar.copy(otr_s[:, 2 * h + dt_, isl], ot_ps[:])

                def emit_oproj(ti):
                    # out rows [128ti,128ti+128): bias + 4 otr-weighted MMs,
                    # then SBUF copy + the output DMA for that half
                    o_ps = zpsp.tile([P, D], F32, tag="z", bufs=2, name=f"o_ps{ti}")
                    nc.tensor.matmul(o_ps[:], one_s[:], bo_s[:], start=True, stop=False)
                    for dt_ in range(4):
                        nc.tensor.matmul(
                            o_ps[:], otr_s[:, dt_, 128 * ti : 128 * (ti + 1)],
                            wob_s[:, dt_, :], start=False, stop=(dt_ == 3),
                        )
                    o_sb = work.tile([P, D], F32, tag="osb", bufs=2, name="o_sb")
                    if ti == 0:
                        nc.scalar.copy(o_sb[:], o_ps[:])
                    else:
                        nc.vector.tensor_copy(o_sb[:], o_ps[:])
                    nc.sync.dma_start(t["out"][128 * ti : 128 * (ti + 1), :], o_sb[:])

                # ---- emission schedule ----
                zp0 = emit_zblock(0)

                # mid gap: projections/scores/exp + blk0 softmax, with blk1
                # T-producers interleaved. The V queue carries ONLY mask/
                # softmax/preT (all PSUM copies go to ACT): any other V op
                # delays blk1's T-production 1:1.
                pre = []
                def preT(n):
                    for _ in range(n):
                        step = len(pre)
                        k, g = step // 4, step % 4
                        pre.append(make_T(128 + 32 * g + k))
                qk_us, v_us = qkv_units()
                for i_, u in enumerate((qk_us[0], qk_us[1], qk_us[4], qk_us[5], qk_us[6], qk_us[7])):
                    u()
                    if i_ % 2:
                        preT(1)
                emit_mask(0, zp0)
                emit_scores(0, 0)
                emit_scores(0, 1)
                softmax_finish(0, 0, zp0)
                emit_transposes(0, 0, "s")
                for i_, u in enumerate((qk_us[2], qk_us[3], qk_us[8], qk_us[9], qk_us[10], qk_us[11])):
                    u()
                    if i_ % 2:
                        preT(1)
                emit_scores(1, 0)
                emit_scores(1, 1)
                softmax_finish(1, 0, zp0)
                preT(6)

                zp1 = emit_zblock(1, pre)

                # tail: V runs mask1 + both ti=1 softmaxes then tp copies;
                # ACT takes v/tp(1,0) copies; PE fills with the deferred V
                # projection + head1/ti=0 transposes + the ti=0 attention
                # half + out rows 0:128 while the ti=1 chain lands last.
                emit_mask(1, zp1)
                softmax_finish(0, 1, zp1)
                softmax_finish(1, 1, zp1)
                emit_transposes(1, 0, "s")
                for u in v_us:
                    u()
                emit_transposes(0, 1, "v")
                emit_transposes(1, 1, "v")
                emit_av(0, 0)
                emit_av(1, 0)
                emit_oproj(0)
                emit_av(0, 1)
                emit_av(1, 1)
                emit_oproj(1)


def _build():
    if "nc" in _STATE:
        return _STATE["nc"]
    nc = bacc.Bacc(
        "TRN2", target_bir_lowering=False, debug=False, enable_asserts=True,
        num_devices=8,
    )
    t = {}
    t["xbT"] = nc.dram_tensor("xbT", [P, 4, S], BF16, kind="ExternalInput").ap()
    t["xlo"] = nc.dram_tensor("xlo", [P, 4, S], BF16, kind="ExternalInput").ap()
    t["wq"] = nc.dram_tensor("wq", [P, 4, D], BF16, kind="ExternalInput").ap()
    t["wk"] = nc.dram_tensor("wk", [P, 4, D], BF16, kind="ExternalInput").ap()
    t["wv"] = nc.dram_tensor("wv", [P, 4, D], BF16, kind="ExternalInput").ap()
    t["wo"] = nc.dram_tensor("wo", [P, 4, D], BF16, kind="ExternalInput").ap()
    t["mqh"] = nc.dram_tensor("mqh", [P, 4, 2 * HID], BF16, kind="ExternalInput").ap()
    t["mql"] = nc.dram_tensor("mql", [P, 4, 2 * HID], BF16, kind="ExternalInput").ap()
    t["constsf"] = nc.dram_tensor("constsf", [P, 68], F32, kind="ExternalInput").ap()
    t["constsb"] = nc.dram_tensor("constsb", [1, D + P], BF16, kind="ExternalInput").ap()
    t["identb"] = nc.dram_tensor("identb", [P, P], BF16, kind="ExternalInput").ap()
    t["out"] = nc.dram_tensor("out", [NCHUNK, D], F32, kind="ExternalOutput").ap()

    with tile.TileContext(nc) as tc:
        _emit(tc, nc, t)
    nc.compile()
    _STATE["nc"] = nc
    return nc


def _prep_in_maps(inputs):
    bf16 = ml_dtypes.bfloat16
    x = np.ascontiguousarray(np.asarray(inputs["x"], np.float32))
    Wq = np.asarray(inputs["Wq"], np.float32)
    Wk = np.asarray(inputs["Wk"], np.float32)
    Wv = np.asarray(inputs["Wv"], np.float32)
    Wo = np.asarray(inputs["Wo"], np.float32)
    bo = np.asarray(inputs["bo"], np.float32)
    W1 = np.asarray(inputs["W1"], np.float64)
    b1 = np.asarray(inputs["b1"], np.float32)
    W2 = np.asarray(inputs["W2"], np.float32)
    b2 = np.asarray(inputs["b2"], np.float32)

    wq_m = 0.5 * (Wq[:, :DH].astype(np.float64) + Wq[:, DH:].astype(np.float64))
    wk_m = 0.5 * (Wk[:, :DH].astype(np.float64) + Wk[:, DH:].astype(np.float64))
    Mq = np.ascontiguousarray((wq_m @ W1[:DH]).astype(np.float32))
    Mk = np.ascontiguousarray((wk_m @ W1[DH:]).astype(np.float32))

    def chunk(a):
        # [D, N] -> [P, 4, N]: partition-chunked layout for one-shot DMA
        return np.ascontiguousarray(a.reshape(4, P, -1).transpose(1, 0, 2))

    constsf = np.zeros((P, 68), np.float32)
    constsf[:, 0] = b1
    constsf[:, 1] = -float(b2[0])
    constsf[:, 2 + 32] = W2[:, 0]          # wsel32 window buffer
    # selv[b][i]=1 if block b's query row i is produced by the DVE 1-op
    # max path (z row then needs the alpha threshold correction)
    for b_, pat in ((0, CONFIG["tpat"]), (1, CONFIG["tpat1"])):
        constsf[:, 66 + b_] = np.array(
            [1.0 if pat[i % 128] == "v" else 0.0 for i in range(P)], np.float32
        )
    constsb = np.zeros((1, D + P), bf16)
    constsb[0, :D] = bo.astype(bf16)
    constsb[0, D:] = np.ones(P, bf16)

    M = np.concatenate([Mq, Mk], axis=1)
    Mh = M.astype(bf16)
    shared = dict(
        wq=chunk((Wq / 16.0).astype(bf16)),
        wk=chunk(Wk.astype(bf16)),
        wv=chunk(Wv.astype(bf16)),
        wo=chunk(Wo.astype(bf16)),
        mqh=chunk(Mh),
        mql=chunk((M - Mh.astype(np.float32)).astype(bf16)),
        constsf=constsf, constsb=constsb,
        identb=np.eye(P, dtype=bf16),
    )
    in_maps = []
    for c in range(8):
        b, i0 = c // 4, (c % 4) * NCHUNK
        m = dict(shared)
        # roll x columns so this core's query chunk sits at j=0..255; the
        # softmax result is invariant to a consistent j-permutation of
        # keys/values/mask, and it lets `a` start from the first x DMA.
        xr = np.roll(x[b].T, -i0, axis=1)
        xh = xr.astype(bf16)
        m["xbT"] = chunk(xh)
        m["xlo"] = chunk((xr - xh.astype(np.float32)).astype(bf16))
        in_maps.append(m)
    return in_maps


def kernel(**inputs):
    nc = _build()
    in_maps = _prep_in_maps(inputs)
    res = run_bass_kernel_spmd(
        nc, in_maps, core_ids=list(range(8)),
        trace=CONFIG["trace"], tmpdir=CONFIG["tmpdir"],
    )
    _STATE["last_result"] = res
    out = np.empty((B, S, D), np.float32)
    for c in range(8):
        b, i0 = c // 4, (c % 4) * NCHUNK
        out[b, i0 : i0 + NCHUNK] = res.results[c]["out"]
    return out

